# revision 10
# baseline (speedup 1.0000x reference)
"""Trainium2 Bass kernel for a GNN message-passing layer.

Reference computation (per graph):
    src,dst = edge_indices
    h   = gelu(concat(x[src], x[dst], e) @ W1m + b1m)          # [E, H]
    msg = h @ W2m + b2m                                        # [E, H]
    agg = segment_sum(msg, dst)                                # [N, H]
    u   = gelu(concat(x, agg) @ W1u + b1u)                     # [N, H]
    out = u @ W2u + b2u                                        # [N, D]

Device strategy (8 cores = 2 graphs x 4 dst-ranges):
  - By linearity, W2m is applied AFTER aggregation: agg = segsum(h) @ W2m + deg*b2m.
  - Per-edge pre-gelu sum s = x[src]@Wsrc + Pdst[dst] + eproj assembled in PSUM:
      * x[src]: host-gathered, streamed in fp8 DoubleRow layout; the src
        projection is an fp8-DR matmul (0.5 cyc/row) accumulated into PSUM.
      * Pdst[dst]: block-local one-hot matmul (lhsT=selTB bf16, rhs=pdst_blk),
        selTB generated on DVE via is_equal(drel_bcast, iota).
      * eproj: fp8 DoubleRow matmul (et and W1e quantized e4m3).
  - gelu reads PSUM directly on ACT (contiguous 4-chunk batches) -> h fp8.
  - Scatter-add via fp8 DoubleRow one-hot matmul: 2 chunks (256 edges) per
    matmul per h-half; sel matrices are exact in fp8; lhsT is a strided view
    of the h tile.
  - Host pre-sorts each graph's edges by destination block (128 nodes/block,
    20 blocks/core); per-slot chunk capacities rounded to even for pairing.
"""

import sys

sys.path.insert(0, "/opt/trn_rl_repo")

import numpy as np
import ml_dtypes

import concourse.bacc as bacc
import concourse.mybir as mybir
import concourse.tile as tile
from concourse.bass_utils import run_bass_kernel_spmd

BF16 = ml_dtypes.bfloat16
FP8 = ml_dtypes.float8_e4m3

B, N, E = 2, 10000, 160000
D, F, H = 128, 64, 256
NCORES = 8
CPG = NCORES // B          # cores per graph = 4
NBLK = 20                  # node blocks per core
BLK = 128                  # nodes per block
NSLICE = NBLK * BLK        # 2560 nodes per core
NPAD = CPG * NSLICE        # 10240 padded nodes per graph

f32 = mybir.dt.float32
bf16 = mybir.dt.bfloat16
fp8 = mybir.dt.float8e4
u8 = mybir.dt.uint8

_BUILD_CACHE = {}


def _build(k_slots):
    """Build the SPMD single-core program.

    k_slots: tuple of NBLK per-block-slot chunk capacities (128 edges/chunk),
    each even.
    """
    k_slots = tuple(int(k) for k in k_slots)
    nchunk = sum(k_slots)
    ecap = nchunk * 128
    coff = [0]
    for k in k_slots:
        coff.append(coff[-1] + k)          # chunk offset per block slot

    nc = bacc.Bacc(None, num_swdge_queues=4)

    # ---- external inputs (per-core) ----
    nfs = nc.dram_tensor("nfs", [D, NSLICE], bf16, kind="ExternalInput")
    xsdr = nc.dram_tensor("xsdr", [64, 2 * ecap], fp8, kind="ExternalInput")
    eftdr = nc.dram_tensor("eftdr", [33, 2 * ecap], fp8, kind="ExternalInput")
    selb8 = nc.dram_tensor("selb8", [128, ecap], fp8, kind="ExternalInput")
    drelb = nc.dram_tensor("drelb", [128, ecap], u8, kind="ExternalInput")
    iotac = nc.dram_tensor("iotac", [128, 512], u8, kind="ExternalInput")
    degrow = nc.dram_tensor("degrow", [1, NSLICE], bf16, kind="ExternalInput")
    wsdr = nc.dram_tensor("wsdr", [64, 2 * H], fp8, kind="ExternalInput")
    wdst = nc.dram_tensor("wdst", [D, H], bf16, kind="ExternalInput")
    w1edr = nc.dram_tensor("w1edr", [33, 2 * H], fp8, kind="ExternalInput")
    # weight k-chunks packed side-by-side: [128, nchunks*cols]
    w2m = nc.dram_tensor("w2m", [128, 2 * H], bf16, kind="ExternalInput")
    b2mr = nc.dram_tensor("b2mr", [1, H], bf16, kind="ExternalInput")
    w1u = nc.dram_tensor("w1u", [128, 3 * H], bf16, kind="ExternalInput")
    b1uc = nc.dram_tensor("b1uc", [128, 2], f32, kind="ExternalInput")
    w2u = nc.dram_tensor("w2u", [128, 2 * D], bf16, kind="ExternalInput")
    b2ur = nc.dram_tensor("b2ur", [1, D], bf16, kind="ExternalInput")
    onesr = nc.dram_tensor("onesr", [1, 128], bf16, kind="ExternalInput")

    out = nc.dram_tensor("out", [NSLICE, D], bf16, kind="ExternalOutput")

    with tile.TileContext(nc) as tc:
        with (
            tc.tile_pool(name="const", bufs=1) as cpool,
            tc.tile_pool(name="xsp", bufs=2) as xsp,
            tc.tile_pool(name="eftp", bufs=2) as eftp,
            tc.tile_pool(name="selp", bufs=2) as selp,
            tc.tile_pool(name="drp", bufs=2) as drp,
            tc.tile_pool(name="stp", bufs=3) as stp,
            tc.tile_pool(name="h8p", bufs=3) as h8p,
            tc.tile_pool(name="cp", bufs=4) as cp,
            tc.tile_pool(name="psA", bufs=3, space="PSUM") as psA,
            tc.tile_pool(name="agg", bufs=2, space="PSUM") as psG,
        ):
            # ---- load constants / persistent tensors into SBUF ----
            def load(dram_t, shape, dtype):
                t = cpool.tile(shape, dtype, tag=dram_t.name)
                nc.sync.dma_start(out=t[:], in_=dram_t[:])
                return t

            nfs_s = load(nfs, [D, NSLICE], bf16)
            iotac_s = load(iotac, [128, 512], u8)
            degrow_s = load(degrow, [1, NSLICE], bf16)
            wsdr_s0 = load(wsdr, [64, 2 * H], fp8)
            wsdr_s = wsdr_s0[:].rearrange("p (a h) -> p a h", a=2)
            wdst_s = load(wdst, [D, H], bf16)
            w1edr_s0 = load(w1edr, [33, 2 * H], fp8)
            w1edr_s = w1edr_s0[:].rearrange("p (a h) -> p a h", a=2)
            w2m_s = load(w2m, [128, 2 * H], bf16)
            b2mr_s = load(b2mr, [1, H], bf16)
            w1u_s = load(w1u, [128, 3 * H], bf16)
            b1uc_s = load(b1uc, [128, 2], f32)
            w2u_s = load(w2u, [128, 2 * D], bf16)
            b2ur_s = load(b2ur, [1, D], bf16)
            onesr_s = load(onesr, [1, 128], bf16)

            # Pdst table lives in SBUF (block-local)
            pdst_sb = cpool.tile([128, NBLK * H], bf16, tag="pdst")
            for nb in range(NBLK):
                ps = psA.tile([128, 1024], f32, tag="psA")
                nc.tensor.matmul(
                    out=ps[:, 0:H], lhsT=nfs_s[:, nb * 128:(nb + 1) * 128],
                    rhs=wdst_s[:], start=True, stop=True,
                )
                nc.scalar.copy(out=pdst_sb[:, nb * H:(nb + 1) * H], in_=ps[:, 0:H])

            # per-512-node-group accumulators (feature-major, bf16)
            NG5 = NSLICE // 512
            aggT = [[cpool.tile([128, 512], bf16, tag=f"aggT{o}_{g}",
                                name=f"aggT{o}_{g}") for g in range(NG5)]
                    for o in range(2)]
            agfT = [[cpool.tile([128, 512], bf16, tag=f"agfT{o}_{g}",
                                name=f"agfT{o}_{g}") for g in range(NG5)]
                    for o in range(2)]
            uT = [[cpool.tile([128, 512], bf16, tag=f"u{o}_{g}",
                              name=f"u{o}_{g}") for g in range(NG5)]
                  for o in range(2)]

            # ---- edge pipeline ----
            for blk in range(NBLK):
                k_blk = k_slots[blk]
                c0blk = coff[blk]          # first chunk index of this block
                e0 = c0blk * 128           # first edge slot of this block
                ew = k_blk * 128
                # host-gathered x[src] in fp8 DoubleRow layout [64, 2, ew]
                xs = xsp.tile([64, 2, ew], fp8, tag="xs")
                nc.sync.dma_start(
                    out=xs[:],
                    in_=xsdr[:].rearrange("p (a e) -> p a e", a=2)[:, :, e0:e0 + ew],
                )
                # edge features in fp8 DoubleRow layout [33, 2, ew]
                et = eftp.tile([33, 2, ew], fp8, tag="eft")
                nc.sync.dma_start(
                    out=et[:],
                    in_=eftdr[:].rearrange("p (a e) -> p a e", a=2)[:, :, e0:e0 + ew],
                )
                # scatter one-hot (fp8) streamed from host
                sb = selp.tile([128, ew], fp8, tag="sel")
                nc.sync.dma_start(out=sb[:], in_=selb8[:, e0:e0 + ew])
                # dst-rel broadcast (uint8) for on-device selTB generation
                db = drp.tile([128, ew], u8, tag="drel")
                nc.sync.dma_start(out=db[:], in_=drelb[:, e0:e0 + ew])

                pdst_blk = pdst_sb[:, blk * H:(blk + 1) * H]
                ag0 = psG.tile([128, 128], f32, tag="agg")
                ag1 = psG.tile([128, 128], f32, tag="agg")
                for g0 in range(0, k_blk, 4):
                    gw = min(4, k_blk - g0)          # 2 or 4 (k_blk even)
                    # selTB for the group: selTB[n, e] = (drel[e] == n)
                    st = stp.tile([128, 512], bf16, tag="seltb")
                    nc.vector.tensor_tensor(
                        out=st[:, 0:gw * 128],
                        in0=db[:, g0 * 128:(g0 + gw) * 128],
                        in1=iotac_s[:, 0:gw * 128],
                        op=mybir.AluOpType.is_equal,
                    )
                    # s = x[src]@Wsrc + Pdst[dst] + eproj, assembled in PSUM
                    pe4 = psA.tile([128, 1024], f32, tag="psA")
                    for ci in range(gw):
                        c = g0 + ci                  # chunk within block
                        osl = slice(ci * H, (ci + 1) * H)
                        nc.tensor.matmul(
                            out=pe4[:, osl],
                            lhsT=xs[:, :, c * 128:(c + 1) * 128],
                            rhs=wsdr_s, start=True, stop=False,
                            perf_mode=mybir.MatmulPerfMode.DoubleRow,
                        )
                        nc.tensor.matmul(
                            out=pe4[:, osl],
                            lhsT=st[:, ci * 128:(ci + 1) * 128],
                            rhs=pdst_blk, start=False, stop=False,
                        )
                        nc.tensor.matmul(
                            out=pe4[:, osl],
                            lhsT=et[:, :, c * 128:(c + 1) * 128],
                            rhs=w1edr_s, start=False, stop=True,
                            perf_mode=mybir.MatmulPerfMode.DoubleRow,
                        )
                    # h = gelu(s): contiguous, same layout as s
                    h8 = h8p.tile([128, 1024], fp8, tag="h8")
                    nc.scalar.activation(
                        out=h8[:, 0:gw * 256], in_=pe4[:, 0:gw * 256],
                        func=mybir.ActivationFunctionType.Gelu_apprx_tanh,
                    )
                    # scatter (feature-major): agg[:, n] += h.T @ sel, 2 chunks
                    # per DR matmul; lhsT is a strided (chunk, h-half) view
                    for pr in range(gw // 2):
                        cpair = g0 + 2 * pr
                        rsl = sb[:, cpair * 128:(cpair + 2) * 128].rearrange(
                            "p (a n) -> p a n", a=2)
                        first = cpair == 0
                        last = cpair + 2 == k_blk
                        hv = h8[:, pr * 512:(pr + 1) * 512].rearrange(
                            "p (sb o h) -> p sb o h", sb=2, o=2)
                        for o, agx in ((0, ag0), (1, ag1)):
                            nc.tensor.matmul(
                                out=agx[:],
                                lhsT=hv[:, :, o:o + 1, :],
                                rhs=rsl,
                                start=first, stop=last,
                                perf_mode=mybir.MatmulPerfMode.DoubleRow,
                            )
                g5, j5 = blk // 4, blk % 4
                csl = slice(j5 * 128, (j5 + 1) * 128)
                nc.vector.tensor_copy(out=aggT[0][g5][:, csl], in_=ag0[:])
                nc.scalar.copy(out=aggT[1][g5][:, csl], in_=ag1[:])

            # ---- stage 3: per-node MLPs (feature-major, 512-node groups) ----
            for g5 in range(NG5):
                sl = slice(g5 * 512, (g5 + 1) * 512)
                for o in range(2):
                    osl = slice(o * 128, (o + 1) * 128)
                    # aggfinal = aggT.T@W2m + deg*b2m   (feature-major out)
                    pa = psA.tile([128, 1024], f32, tag="psA")
                    nc.tensor.matmul(out=pa[:, 0:512], lhsT=w2m_s[:, 0 * H + o * 128:0 * H + (o + 1) * 128],
                                     rhs=aggT[0][g5][:], start=True, stop=False)
                    nc.tensor.matmul(out=pa[:, 0:512], lhsT=w2m_s[:, 1 * H + o * 128:1 * H + (o + 1) * 128],
                                     rhs=aggT[1][g5][:], start=False, stop=False)
                    nc.tensor.matmul(out=pa[:, 0:512], lhsT=b2mr_s[:, osl],
                                     rhs=degrow_s[:, sl], start=False, stop=True)
                    nc.vector.tensor_copy(out=agfT[o][g5][:], in_=pa[:, 0:512])
                for o in range(2):
                    # u = gelu(concat(x, aggfinal) @ W1u + b1u)
                    pu = psA.tile([128, 1024], f32, tag="psA")
                    nc.tensor.matmul(out=pu[:, 0:512], lhsT=w1u_s[:, 0 * H + o * 128:0 * H + (o + 1) * 128],
                                     rhs=nfs_s[:, sl], start=True, stop=False)
                    nc.tensor.matmul(out=pu[:, 0:512], lhsT=w1u_s[:, 1 * H + o * 128:1 * H + (o + 1) * 128],
                                     rhs=agfT[0][g5][:], start=False, stop=False)
                    nc.tensor.matmul(out=pu[:, 0:512], lhsT=w1u_s[:, 2 * H + o * 128:2 * H + (o + 1) * 128],
                                     rhs=agfT[1][g5][:], start=False, stop=True)
                    nc.scalar.activation(
                        out=uT[o][g5][:], in_=pu[:, 0:512],
                        func=mybir.ActivationFunctionType.Gelu_apprx_tanh,
                        bias=b1uc_s[:, o:o + 1],
                    )
                # out = u @ W2u + b2u   (token-major out per node block)
                for j5 in range(4):
                    blk = g5 * 4 + j5
                    csl = slice(blk * 128, (blk + 1) * 128)
                    jsl = slice(j5 * 128, (j5 + 1) * 128)
                    po = psA.tile([128, 1024], f32, tag="psA")
                    nc.tensor.matmul(out=po[:, 0:128], lhsT=uT[0][g5][:, jsl], rhs=w2u_s[:, 0:D],
                                     start=True, stop=False)
                    nc.tensor.matmul(out=po[:, 0:128], lhsT=uT[1][g5][:, jsl], rhs=w2u_s[:, D:2 * D],
                                     start=False, stop=False)
                    nc.tensor.matmul(out=po[:, 0:128], lhsT=onesr_s[:], rhs=b2ur_s[:],
                                     start=False, stop=True)
                    oc = cp.tile([128, 128], bf16, tag="ocp")
                    nc.vector.tensor_copy(out=oc[:], in_=po[:, 0:128])
                    nc.sync.dma_start(out=out[csl, :], in_=oc[:])

    nc.finalize()
    return nc


def _core_block_counts(g, r, edge_indices):
    dst = edge_indices[g, :, 1]
    lo, hi = r * NSLICE, (r + 1) * NSLICE
    mask = (dst >= lo) & (dst < hi)
    dloc = dst[mask] - lo
    return np.bincount(dloc // BLK, minlength=NBLK)


def _prep_core_inputs(g, r, perm, k_slots, edge_indices, edge_features, shared):
    """Host-side shard prep for core (graph g, dst-range r).

    perm: perm[slot] = original block id placed at this slot.
    k_slots: per-slot chunk capacities (shared across cores, even).
    """
    k_slots = [int(k) for k in k_slots]
    nchunk = sum(k_slots)
    ecap = nchunk * 128
    coff = np.concatenate([[0], np.cumsum(k_slots)]).astype(np.int64)

    dst = edge_indices[g, :, 1]
    src = edge_indices[g, :, 0]
    lo, hi = r * NSLICE, (r + 1) * NSLICE

    mask = (dst >= lo) & (dst < hi)
    eid = np.nonzero(mask)[0]
    dloc = dst[eid] - lo
    blk_of = dloc // BLK
    # map original block -> slot
    slot_of_blk = np.empty(NBLK, dtype=np.int64)
    slot_of_blk[perm] = np.arange(NBLK)
    slot_id = slot_of_blk[blk_of]
    order = np.argsort(slot_id, kind="stable")
    eid = eid[order]
    dloc = dloc[order]
    slot_id = slot_id[order]
    counts = np.bincount(slot_id, minlength=NBLK)

    slot = np.zeros(ecap, dtype=np.int64) - 1
    srcpad = np.zeros(ecap, dtype=np.int64)            # source node (0 for pads)
    drel = np.full(ecap, 255, dtype=np.int64)          # 255 => sel column all-zero
    epos = 0
    for s in range(NBLK):
        cnt = counts[s]
        assert cnt <= k_slots[s] * 128
        s0 = int(coff[s]) * 128
        ids = eid[epos:epos + cnt]
        srcpad[s0:s0 + cnt] = src[ids]
        drel[s0:s0 + cnt] = dloc[epos:epos + cnt] % BLK
        slot[s0:s0 + cnt] = ids
        epos += cnt

    # host-gathered x[src] in fp8 DoubleRow layout [64, 2, ecap]
    xsrc = shared["_xf8g"][g][:, srcpad]               # [128, ecap] fp8
    xsdr_np = np.ascontiguousarray(
        xsrc.reshape(2, 64, ecap).transpose(1, 0, 2).reshape(64, 2 * ecap))

    # edge features in fp8 DoubleRow layout: rows 0..63 feats, 64 ones, 65 zero
    etfull = np.zeros((66, ecap), dtype=np.float32)
    valid = slot >= 0
    etfull[:F, valid] = edge_features[g, slot[valid], :].T
    etfull[F, :] = 1.0
    eftdr_np = np.ascontiguousarray(
        etfull.reshape(2, 33, ecap).transpose(1, 0, 2).reshape(33, 2 * ecap)
    ).astype(FP8)

    # scatter one-hot: selb8[p, c*128+n] = (drel[c*128+p] == n), fp8
    dr2 = drel.reshape(nchunk, 128)                         # [c, p]
    selb_np = (dr2[:, :, None] == np.arange(128)[None, None, :])
    selb_np = np.ascontiguousarray(
        selb_np.transpose(1, 0, 2).reshape(128, ecap)).astype(FP8)

    # drel broadcast across partitions (uint8)
    drelb_np = np.ascontiguousarray(
        np.broadcast_to(drel.astype(np.uint8)[None, :], (128, ecap)))

    # degrees / node features in slot order
    deg = np.bincount(dloc, minlength=NSLICE).astype(np.float64)
    deg_slot = deg.reshape(NBLK, BLK)[perm].reshape(-1)
    degc = deg_slot[None, :].astype(BF16)

    nfs_full = shared["_nftg"][g][:, lo:hi]                       # [D, NSLICE]
    nfs_slot = np.ascontiguousarray(
        nfs_full.reshape(D, NBLK, BLK)[:, perm, :].reshape(D, NSLICE))

    inp = dict(shared)
    inp["nfs"] = nfs_slot
    inp["xsdr"] = xsdr_np
    inp["eftdr"] = eftdr_np
    inp["selb8"] = selb_np
    inp["drelb"] = drelb_np
    inp["degrow"] = degc
    return {k: v for k, v in inp.items() if not k.startswith("_")}


def kernel(node_features, edge_indices, edge_features,
           W1m, b1m, W2m, b2m, W1u, b1u, W2u, b2u):
    node_features = np.asarray(node_features)
    edge_indices = np.asarray(edge_indices)
    edge_features = np.asarray(edge_features)

    # per-core block permutations (descending count) and shared slot capacities
    perms = []
    sorted_counts = np.zeros((NCORES, NBLK), dtype=np.int64)
    for c in range(NCORES):
        g, r = c // CPG, c % CPG
        counts = _core_block_counts(g, r, edge_indices)
        perm = np.argsort(-counts, kind="stable")
        perms.append(perm)
        sorted_counts[c] = counts[perm]
    k_raw = np.ceil(sorted_counts.max(axis=0) / 128.0).astype(np.int64)
    k_slots = tuple(int(v + (v & 1)) for v in k_raw)    # round up to even

    if k_slots not in _BUILD_CACHE:
        _BUILD_CACHE[k_slots] = _build(k_slots)
    nc = _BUILD_CACHE[k_slots]

    # node features: feature-major bf16 (pdst/u stages) and fp8 (src stream)
    nftg = np.zeros((B, D, NPAD), dtype=BF16)
    xf8g = np.zeros((B, 128, N), dtype=FP8)
    for g in range(B):
        xg = np.asarray(node_features[g]).astype(BF16)       # [N, D]
        nftg[g, :, :N] = xg.T
        xf8g[g] = xg.T.astype(FP8)

    w1efull = np.zeros((66, H), dtype=np.float32)
    w1efull[:F] = np.asarray(W1m)[2 * D:]
    w1efull[F] = np.asarray(b1m)
    w1edr_np = np.ascontiguousarray(
        w1efull.reshape(2, 33, H).transpose(1, 0, 2).reshape(33, 2 * H)).astype(FP8)
    wsdr_np = np.ascontiguousarray(
        np.asarray(W1m)[:D].astype(np.float32)
        .reshape(2, 64, H).transpose(1, 0, 2).reshape(64, 2 * H)).astype(FP8)

    shared = {
        "_nftg": nftg,
        "_xf8g": xf8g,
        "wsdr": wsdr_np,
        "wdst": np.asarray(W1m)[D:2 * D].astype(BF16),
        "w1edr": w1edr_np,
        "iotac": np.ascontiguousarray(
            np.broadcast_to(np.arange(128, dtype=np.uint8)[:, None], (128, 512))),
        "w2m": np.asarray(W2m).reshape(2, 128, H).transpose(1, 0, 2).reshape(128, 2 * H).astype(BF16),
        "b2mr": np.asarray(b2m)[None, :].astype(BF16),
        "w1u": np.asarray(W1u).reshape(3, 128, H).transpose(1, 0, 2).reshape(128, 3 * H).astype(BF16),
        "b1uc": np.asarray(b1u).reshape(2, 128).T.astype(np.float32).copy(),
        "w2u": np.asarray(W2u).reshape(2, 128, D).transpose(1, 0, 2).reshape(128, 2 * D).astype(BF16),
        "b2ur": np.asarray(b2u)[None, :].astype(BF16),
        "onesr": np.ones((1, 128), dtype=BF16),
    }

    in_maps = []
    for c in range(NCORES):
        g, r = c // CPG, c % CPG
        in_maps.append(_prep_core_inputs(
            g, r, perms[c], k_slots, edge_indices, edge_features, shared))

    global _LAST_IN_MAPS
    _LAST_IN_MAPS = in_maps
    res = run_bass_kernel_spmd(nc, in_maps, core_ids=list(range(NCORES)))

    outp = np.zeros((B, NPAD, D), dtype=np.float32)
    for c in range(NCORES):
        g, r = c // CPG, c % CPG
        o = res.results[c]["out"].astype(np.float32)   # [NSLICE, D] slot-major
        inv = np.empty(NBLK, dtype=np.int64)
        inv[perms[c]] = np.arange(NBLK)
        o_blocks = o.reshape(NBLK, BLK, D)[inv]        # back to block order
        outp[g, r * NSLICE:(r + 1) * NSLICE, :] = o_blocks.reshape(NSLICE, D)
    return outp[:, :N, :]
